# revision 4
# baseline (speedup 1.0000x reference)
"""Trainium2 Bass kernel for CaiT talking-heads attention, v2.

B=8 batch, N=1024 tokens, DIM=512, 8 heads x 64. Data-parallel: one batch
element per NeuronCore (8 cores).

v2 strategy (vs v1 which folded both head-mixes into 8x-wide matmuls):
  - raw per-head scores S_h at contraction 64 (1 PE pass, not 4)
  - the [8x8] head mixes run as real 128x128 block-diagonal matmuls in a
    (slot s, i_lo)-interleaved partition layout (1 PE pass each)
  - partition<->free shuffles via PE transposes (the DMA xbar corrupts even
    elements of later calls on this runtime, so only the initial x^T uses it)
  - softmax normalization folded into the post-mix lhsT (scaled per-tile by
    1/rowsum); rowsum comes free from the exp activation's accum_out
  - PV contracts j at width-64 per mixed head into transposed output OT[c,i],
    which feeds the final projection without further transposes

bf16 on the PE except the final projection (fp32r).

Layouts (chunk = 128 i-tokens, i_g = 16 i-tokens, slot s: s<4 -> head 2s at
partition-offset 0, s>=4 -> head 2(s-4)+1 at offset 64):
  xT   [c 128, fc 4, tok 1024]            (bf16, xbar once at start)
  QT/KT[c 128, cc 4, tok 1024]            (bf16)
  V    [tok 128, jb 8, c 512]             (bf16)
  S_sb [j 128, jb 8, i_g 8, i_lo 16, s 8] (bf16) per chunk
  T1   [(i_lo*8+s) 128, i_g 8, jb 8, j 128] (bf16) per chunk (PE transpose)
  D/P  [(g*16+i_lo) 128, j 512 halves]    premix out / exp
  Ap   [(G*16+i_lo) 128, i_g 8, j 1024]   (bf16) per chunk
  AT   [j 128, jb 8, i_g 8, q 128]        (bf16) per chunk (PE transpose)
  OT   [c 128, k 4, i 128]                (fp32r) per chunk, from PV psum
"""

import os

import numpy as np
import ml_dtypes

import concourse.bass as bass
import concourse.bacc as bacc
import concourse.mybir as mybir
from concourse.bass_utils import run_bass_kernel_spmd
from concourse.masks import make_identity
from concourse.tile import TileContext

P = 128
N = 1024
DIM = 512
H = 8
DH = 64
F32 = mybir.dt.float32
F32R = mybir.dt.float32r
BF16 = mybir.dt.bfloat16

IB = N // P     # 8 token blocks (jb)
CC = DIM // P   # 4 channel chunks
NCH = IB        # 8 i-chunks of 128
NIG = 8         # i_g groups per chunk (16 i each)
NCORES = 8
EXPFN = mybir.ActivationFunctionType.Exp

bf16 = ml_dtypes.bfloat16

DBG_NCH = int(os.environ.get("K2_NCH", NCH))


def build_bass():
    nc = bacc.Bacc("TRN2")

    x_d = nc.dram_tensor("x", [N, DIM], BF16, kind="ExternalInput")
    wq_d = nc.dram_tensor("wq", [DIM, DIM], BF16, kind="ExternalInput")
    wk_d = nc.dram_tensor("wk", [DIM, DIM], BF16, kind="ExternalInput")
    wv_d = nc.dram_tensor("wv", [DIM, DIM], BF16, kind="ExternalInput")
    wout_d = nc.dram_tensor("wout", [DIM, DIM], F32R, kind="ExternalInput")
    # mpre[i_lo*8+s, g*16+i_lo'] = (i_lo==i_lo') * mix_pre[h(s),g] / 8
    mpre_d = nc.dram_tensor("mpre", [P, P], BF16, kind="ExternalInput")
    # mpost[g*16+i_lo, G*16+i_lo'] = (i_lo==i_lo') * mix_post[g,G]
    mpost_d = nc.dram_tensor("mpost", [P, P], BF16, kind="ExternalInput")
    bias_d = nc.dram_tensor("biasb", [P, DIM], F32R, kind="ExternalInput")
    y_d = nc.dram_tensor("y", [N, DIM], F32R, kind="ExternalOutput")

    with TileContext(nc) as tc:
        with tc.tile_pool(name="persist", bufs=1) as pp:
            ident0 = pp.tile([P, P], F32)
            make_identity(nc, ident0)
            ident = pp.tile([P, P], BF16)
            nc.vector.tensor_copy(ident[:], ident0[:])
            mpre = pp.tile([P, P], BF16)
            nc.sync.dma_start(mpre[:], mpre_d[:])
            mpost = pp.tile([P, P], BF16)
            nc.sync.dma_start(mpost[:], mpost_d[:])
            bias = pp.tile([P, DIM], F32R)
            nc.sync.dma_start(bias[:], bias_d[:])
            wout = pp.tile([P, CC, DIM], F32R)
            for c in range(CC):
                nc.sync.dma_start(wout[:, c, :], wout_d[c * P:(c + 1) * P, :])

            QT = pp.tile([P, CC, N], BF16)   # QT[p,cc,i] = q[i, cc*128+p]
            KT = pp.tile([P, CC, N], BF16)
            V = pp.tile([P, IB, DIM], BF16)  # V[p,jb,c] = v[jb*128+p, c]

            # ---- phase 0/1: x load, xbar transpose, projections ----
            with tc.tile_pool(name="ph01", bufs=1) as p01:
                xsb = p01.tile([P, IB, DIM], BF16)
                for b in range(IB):
                    eng = (nc.sync, nc.scalar)[b % 2]
                    eng.dma_start(xsb[:, b, :], x_d[b * P:(b + 1) * P, :])
                wq = p01.tile([P, CC, DIM], BF16)
                wk = p01.tile([P, CC, DIM], BF16)
                wv = pp.tile([P, CC, DIM], BF16)
                for c in range(CC):
                    nc.sync.dma_start(wq[:, c, :], wq_d[c * P:(c + 1) * P, :])
                    nc.scalar.dma_start(wk[:, c, :], wk_d[c * P:(c + 1) * P, :])
                    nc.sync.dma_start(wv[:, c, :], wv_d[c * P:(c + 1) * P, :])
                xT = pp.tile([P, CC, N], BF16)  # xT[p,fc,i] = x[i, fc*128+p]
                for b in range(IB):
                    # out[p',fc,q] = in[q, fc*128+p'] = x[b*128+q, fc*128+p']
                    nc.sync.dma_start_transpose(
                        xT[:, :, b * P:(b + 1) * P], xsb[:, b, :]
                    )


                with tc.tile_pool(name="ps01", bufs=4, space="PSUM") as psp:
                    for cc in range(CC):
                        for ih in range(2):
                            isl = slice(ih * 512, (ih + 1) * 512)
                            pq = psp.tile([P, DIM], F32, tag="ps")
                            pk = psp.tile([P, DIM], F32, tag="ps")
                            for fc in range(CC):
                                nc.tensor.matmul(
                                    pq, wq[:, fc, cc * P:(cc + 1) * P],
                                    xT[:, fc, isl],
                                    start=(fc == 0), stop=(fc == CC - 1),
                                )
                            for fc in range(CC):
                                nc.tensor.matmul(
                                    pk, wk[:, fc, cc * P:(cc + 1) * P],
                                    xT[:, fc, isl],
                                    start=(fc == 0), stop=(fc == CC - 1),
                                )
                            nc.vector.tensor_copy(QT[:, cc, isl], pq)
                            nc.scalar.copy(KT[:, cc, isl], pk)
                    for jb in range(IB):
                        pv = psp.tile([P, DIM], F32, tag="ps")
                        for fc in range(CC):
                            nc.tensor.matmul(
                                pv, xT[:, fc, jb * P:(jb + 1) * P],
                                wv[:, fc, :],
                                start=(fc == 0), stop=(fc == CC - 1),
                            )
                        nc.scalar.copy(V[:, jb, :], pv)

            # ---- phase 2: per i-chunk of 128 tokens ----
            with (
                tc.tile_pool(name="ssb", bufs=2) as p_s,
                tc.tile_pool(name="t1b", bufs=2) as p_t1,
                tc.tile_pool(name="pb", bufs=4) as p_p,
                tc.tile_pool(name="atb", bufs=2) as p_at,
                tc.tile_pool(name="otb", bufs=2) as p_ot,
                tc.tile_pool(name="ysb", bufs=2) as p_y,
                tc.tile_pool(name="small", bufs=12) as p_sm,
                tc.tile_pool(name="ps_s", bufs=2, space="PSUM") as ps_s,
                tc.tile_pool(name="ps_t", bufs=2, space="PSUM") as ps_t,
                tc.tile_pool(name="ps_d", bufs=1, space="PSUM") as ps_d,
                tc.tile_pool(name="ps_am", bufs=2, space="PSUM") as ps_am,
            ):
                ps_pv = ps_s
                pending = []  # (ch, AT) awaiting PV + projection

                def emit_tail(ch_p, AT_p):
                    csl_p = slice(ch_p * P, (ch_p + 1) * P)
                    OT = p_ot.tile([P, CC, P], F32R, tag="ot")
                    for G in range(H):
                        pv = ps_t.tile([DH, P], F32, tag="t")
                        for jb in range(IB):
                            nc.tensor.matmul(
                                pv,
                                V[:, jb, G * DH:(G + 1) * DH],
                                AT_p[:, jb, :, G * 16:(G + 1) * 16],
                                start=(jb == 0), stop=(jb == IB - 1),
                            )
                        po = (G % 2) * DH
                        nc.vector.tensor_copy(OT[po:po + DH, G // 2, :], pv)
                    py = ps_d.tile([P, DIM], F32, tag="d")
                    for k in range(CC):
                        nc.tensor.matmul(
                            py, OT[:, k, :], wout[:, k, :],
                            start=(k == 0), stop=(k == CC - 1),
                        )
                    ysb = p_y.tile([P, DIM], F32R, tag="ysb")
                    nc.vector.tensor_add(out=ysb[:], in0=py, in1=bias)
                    nc.sync.dma_start(y_d[csl_p, :], ysb)

                for ch in range(DBG_NCH):
                    csl = slice(ch * P, (ch + 1) * P)
                    # S_sb[j, jb, i_g, i_lo, s]
                    S_sb = p_s.tile([P, IB, NIG, 16, H], BF16, tag="s")
                    for jb in range(IB):
                        for par in range(2):
                            ps = ps_s.tile([P, 4, NIG, 16], F32, tag="ps")
                            po = par * DH
                            for cc in range(4):
                                nc.tensor.matmul(
                                    ps[:, cc, :, :],
                                    KT[po:po + DH, cc, jb * P:(jb + 1) * P],
                                    QT[po:po + DH, cc, csl],
                                    start=True, stop=True,
                                )
                            # (s_lo, i_g, i_lo) -> (i_g, i_lo, s_lo)
                            nc.vector.tensor_copy(
                                S_sb[:, jb, :, :, par * 4:(par + 1) * 4],
                                ps.transpose([0, 2, 3, 1]),
                            )

                    # T1[(i_lo,s), i_g, jb, j] via PE transposes, 4 jb a tile
                    T1 = p_t1.tile([P, NIG, IB, P], BF16, tag="t1")
                    for ig in range(NIG):
                        for jh in range(2):
                            pt = ps_t.tile([P, 4, P], BF16, tag="t")
                            for jl in range(4):
                                jb = jh * 4 + jl
                                nc.tensor.transpose(
                                    pt[:, jl, :],
                                    S_sb[:, jb, ig, :, :],
                                    ident,
                                )
                            nc.vector.tensor_copy(
                                T1[:, ig, jh * 4:(jh + 1) * 4, :], pt[:, :, :]
                            )

                    # mixes + exp, per i_g, j in halves of 512
                    # AT[j, jb, i_g, q]
                    AT = p_at.tile([P, IB, NIG, P], BF16, tag="at")
                    for ig in range(NIG):
                        Pt = p_p.tile([P, N], BF16, tag="p")
                        dm = ps_d.tile([P, N], F32, tag="d")
                        for jh in range(2):
                            nc.tensor.matmul(
                                dm[:, jh * 512:(jh + 1) * 512], mpre,
                                T1[:, ig, jh * 4:(jh + 1) * 4, :],
                                start=True, stop=True,
                            )
                        rs = p_sm.tile([P, 1], F32, tag="rsum")
                        nc.scalar.activation(
                            Pt[:], dm[:], EXPFN, accum_out=rs[:]
                        )
                        rr = p_sm.tile([P, 1], F32, tag="rr")
                        nc.vector.reciprocal(rr, rs)
                        smix = p_sm.tile([P, P], BF16, tag="smix")
                        nc.vector.tensor_scalar_mul(smix, mpost, rr)
                        # postmix transposed: AT-block[j, q] = sum_k
                        # P[k, j] * smix[k, q]; P slice is lhsT, smix is rhs
                        for jh in range(2):
                            am = ps_am.tile([P, 4, P], F32, tag="am")
                            for jl in range(4):
                                jb = jh * 4 + jl
                                nc.tensor.matmul(
                                    am[:, jl, :],
                                    Pt[:, jb * P:(jb + 1) * P],
                                    smix,
                                    start=True, stop=True,
                                )
                            nc.scalar.copy(
                                AT[:, jh * 4:(jh + 1) * 4, ig, :], am[:, :, :]
                            )

                    # PV + projection for the PREVIOUS chunk (software
                    # pipelining: keeps next chunk's copies ahead of this
                    # chunk's tail in every engine queue)
                    pending.append((ch, AT))
                    if len(pending) > 1:
                        ch_p, AT_p = pending.pop(0)
                        emit_tail(ch_p, AT_p)

                for ch_p, AT_p in pending:
                    emit_tail(ch_p, AT_p)

    nc.finalize()
    return nc


_NC_CACHE = None
TRACE = False
LAST_RESULT = None


def kernel(x, w_q, w_kv, mix_pre, mix_post, w_out, b_out):
    global _NC_CACHE
    x = np.asarray(x, np.float32)
    w_q = np.asarray(w_q, np.float32)
    w_kv = np.asarray(w_kv, np.float32)
    mix_pre = np.asarray(mix_pre, np.float32)
    mix_post = np.asarray(mix_post, np.float32)
    w_out = np.asarray(w_out, np.float32)
    b_out = np.asarray(b_out, np.float32)

    w_k = np.ascontiguousarray(w_kv[:, :DIM])
    w_v = np.ascontiguousarray(w_kv[:, DIM:])

    # block-diagonal-in-i_lo mix matrices, 16-way interleaved.
    # slot s at partition i_lo*8+s holds head h = 2s (s<4) / 2(s-4)+1 (s>=4)
    il = np.arange(16)
    mpre = np.zeros((P, P), np.float32)
    mpost = np.zeros((P, P), np.float32)
    for s in range(H):
        h = 2 * s if s < 4 else 2 * (s - 4) + 1
        for g in range(H):
            mpre[il * 8 + s, g * 16 + il] = mix_pre[h, g] * 0.125
    for g in range(H):
        for G in range(H):
            mpost[g * 16 + il, G * 16 + il] = mix_post[g, G]
    biasb = np.broadcast_to(b_out[None, :], (P, DIM)).astype(np.float32).copy()

    if _NC_CACHE is None:
        _NC_CACHE = build_bass()
    nc = _NC_CACHE

    base = {
        "wq": w_q.astype(bf16), "wk": w_k.astype(bf16),
        "wv": w_v.astype(bf16), "wout": w_out,
        "mpre": mpre.astype(bf16), "mpost": mpost.astype(bf16),
        "biasb": biasb,
    }
    in_maps = [
        dict(base, x=np.ascontiguousarray(x[b]).astype(bf16))
        for b in range(NCORES)
    ]
    global LAST_RESULT
    res = run_bass_kernel_spmd(
        nc, in_maps, core_ids=list(range(NCORES)), trace=TRACE,
        trace_cores=list(range(NCORES)) if TRACE else None,
    )
    LAST_RESULT = res
    out = np.stack([res.results[b]["y"] for b in range(NCORES)], axis=0)
    return out.astype(np.float32)
